# revision 9
# baseline (speedup 1.0000x reference)
"""Trainium2 Bass SPMD kernel for nn_BaseCrystalModel (SchNet-style GNN).

Self-contained: host-side sharding/padding/weight-folding + Bass program
builder + SPMD runner over 8 NeuronCores.

Decomposition (per core, graph/data parallel):
  - nodes sharded contiguously at graph boundaries (128 graphs/core), padded
    to N_pad (multiple of 512); edges assigned to the core owning their dst,
    sorted by dst block (128 nodes), padded to U chunks of 128 edges/block.
  - embed + BN sharded; BN statistics via a tiny AllReduce.
  - per layer: hb = h @ Wi_in computed for the local slice, AllGathered into
    a full [N_eff, C] table; hin gather via indirect DMA; edge filters via
    PE matmuls (channel-major); segment-sum scatter via one-hot matmuls
    accumulating in PSUM per 128-node block.
  - all biases and the shifted-softplus -log(2) terms are folded exactly
    into per-channel constants on the host:
      eps_i  = bi_in[i] @ Wi_in[i]^-1        (hidden-state shift convention)
      delta_i = (be2[i] - ln2*colsum(We2[i])) @ We2[i]^-1
      const_i = eps_{i+1} - eps_i - ln2
  - env (cosine cutoff) is folded into the scatter one-hot indicator.
  - readout pools h @ W_r1 per graph via one-hot matmuls; tiny MLP on-chip.
"""
import numpy as np
from contextlib import ExitStack

import concourse.bass as bass
import concourse.bacc as bacc
import concourse.mybir as mybir
import concourse.tile as tile
from concourse.bass import IndirectOffsetOnAxis
from concourse.bass_utils import run_bass_kernel_spmd

F32 = mybir.dt.float32
I32 = mybir.dt.int32
AF = mybir.ActivationFunctionType
ALU = mybir.AluOpType
LOG2 = float(np.log(2.0))
BN_EPS = 1e-5
CUTOFF = 10.0
N_CORES = 8
G_TOTAL = 1024  # num graphs in the reference model (constant in reference.py)


# --------------------------------------------------------------------------
# host-side preprocessing
# --------------------------------------------------------------------------

def _prep(inputs, n_cores=N_CORES, n_graphs=None):
    x = np.asarray(inputs["x"], np.float32)
    edge_index = np.asarray(inputs["edge_index"])
    ew = np.asarray(inputs["edge_weight"], np.float32)
    ea = np.asarray(inputs["edge_attr"], np.float32)
    batch = np.asarray(inputs["batch"]).astype(np.int64)

    N, FIN = x.shape
    E = edge_index.shape[1]
    C = int(np.asarray(inputs["W_l2"]).shape[0])
    L = int(np.asarray(inputs["Wi_in"]).shape[0])
    G = n_graphs if n_graphs is not None else G_TOTAL
    Gp = ((G + n_cores - 1) // n_cores) * n_cores
    g_per_core = Gp // n_cores

    src = edge_index[0].astype(np.int64)
    dst = edge_index[1].astype(np.int64)

    bounds = np.searchsorted(batch, np.arange(0, Gp + 1, g_per_core))
    starts, ends = bounds[:-1], bounds[1:]
    n_loc = ends - starts
    N_pad = int(((n_loc.max() + 511) // 512) * 512)
    R = N_pad // 128
    N_eff = n_cores * N_pad

    pos = np.zeros(N, np.int64)
    for k in range(n_cores):
        idx = np.arange(starts[k], ends[k])
        pos[idx] = k * N_pad + (idx - starts[k])
    core_of_node = np.searchsorted(ends, np.arange(N), side="right")

    W_l1 = np.asarray(inputs["W_l1"], np.float64)
    b_l1 = np.asarray(inputs["b_l1"], np.float64)
    Wi_in = np.asarray(inputs["Wi_in"], np.float64)
    bi_in = np.asarray(inputs["bi_in"], np.float64)
    We1 = np.asarray(inputs["We1"], np.float64)
    be1 = np.asarray(inputs["be1"], np.float64)
    We2 = np.asarray(inputs["We2"], np.float64)
    be2 = np.asarray(inputs["be2"], np.float64)

    W_l1_aug = np.concatenate([W_l1, b_l1[None, :]], 0)
    We1_aug = np.stack(
        [np.concatenate([We1[i], be1[i][None, :]], 0) for i in range(L)])

    eps = np.zeros((L + 1, C))
    for i in range(L):
        eps[i] = np.linalg.solve(Wi_in[i].T, bi_in[i])
    delta = np.zeros((L, C))
    for i in range(L):
        beta = be2[i] - LOG2 * We2[i].sum(0)
        delta[i] = np.linalg.solve(We2[i].T, beta)
    const = np.stack([eps[i + 1] - eps[i] - LOG2 for i in range(L)])

    env = 0.5 * (np.cos(ew.astype(np.float64) * (np.pi / CUTOFF)) + 1.0)

    dst_core = core_of_node[dst]
    dst_pos = pos[dst]
    U = 1
    percore_edges = []
    for k in range(n_cores):
        e_idx = np.nonzero(dst_core == k)[0]
        dloc = dst_pos[e_idx] - k * N_pad
        blk = dloc // 128
        order = np.argsort(blk, kind="stable")
        e_idx = e_idx[order]
        cnt = np.bincount(blk[order], minlength=R)
        U = max(U, int((cnt.max() + 127) // 128))
        percore_edges.append((e_idx, cnt))
    E_pad = R * U * 128

    meta = dict(N=N, FIN=FIN, E=E, C=C, L=L, G=G, Gp=Gp,
                g_per_core=g_per_core, n_cores=n_cores, N_pad=N_pad, R=R,
                U=U, E_pad=E_pad, N_eff=N_eff)

    ea_aug = np.concatenate(
        [ea.astype(np.float64), np.ones((E, 1))], 1)  # [E, 4]

    common = dict(
        W_l1_aug=W_l1_aug.astype(np.float32),
        W_l2=np.asarray(inputs["W_l2"], np.float32),
        bn_g=np.asarray(inputs["bn_g"], np.float32).reshape(C, 1),
        bn_b=np.asarray(inputs["bn_b"], np.float32).reshape(C, 1),
        eps1=eps[0].astype(np.float32).reshape(C, 1),
        Wi_in=np.asarray(Wi_in, np.float32),
        We1_aug=We1_aug.astype(np.float32),
        We2=np.asarray(We2, np.float32),
        Wi_out=np.asarray(inputs["Wi_out"], np.float32),
        bi_out=np.asarray(inputs["bi_out"], np.float32).reshape(L, C, 1),
        delta=delta.astype(np.float32).reshape(L, C, 1),
        const=const.astype(np.float32).reshape(L, C, 1),
        W_r1=np.asarray(inputs["W_r1"], np.float32),
        b_r1=np.asarray(inputs["b_r1"], np.float32).reshape(-1, 1),
        W_r2=np.asarray(inputs["W_r2"], np.float32),
        b_r2=np.asarray(inputs["b_r2"], np.float32).reshape(-1, 1),
    )

    percore = []
    for k in range(n_cores):
        e_idx, cnt = percore_edges[k]
        ea_t = np.zeros((4, E_pad), np.float32)
        src_idx = np.zeros((R, 128, U), np.int32)
        ind = np.zeros((R * U, 128, 128), np.float32)
        ptr = 0
        for b in range(R):
            nb = int(cnt[b])
            eb = e_idx[ptr:ptr + nb]
            ptr += nb
            dloc = (dst_pos[eb] - k * N_pad - b * 128).astype(np.int64)
            nchunk = (nb + 127) // 128
            for j in range(nchunk):
                ee = eb[j * 128:(j + 1) * 128]
                dd = dloc[j * 128:(j + 1) * 128]
                ne = len(ee)
                ch = b * U + j
                ea_t[:, ch * 128:ch * 128 + ne] = ea_aug[ee].T
                src_idx[b, :ne, j] = pos[src[ee]]
                ind[ch, np.arange(ne), dd] = env[ee]
        xs = np.zeros((FIN + 1, N_pad), np.float32)
        nl = int(n_loc[k])
        xs[:FIN, :nl] = x[starts[k]:ends[k]].T
        xs[FIN, :nl] = 1.0
        g_ind = np.zeros((R, 128, g_per_core), np.float32)
        gl = batch[starts[k]:ends[k]] - k * g_per_core
        g_ind[np.arange(nl) // 128, np.arange(nl) % 128, gl] = 1.0
        d = dict(x_slice=xs, ea_t=ea_t, src_idx=src_idx, ind=ind, g_ind=g_ind)
        d.update(common)
        percore.append(d)
    return meta, percore


# --------------------------------------------------------------------------
# bass program
# --------------------------------------------------------------------------

def _build(meta, debug_taps=False):
    C, L, FIN = meta["C"], meta["L"], meta["FIN"]
    N_pad, R, U = meta["N_pad"], meta["R"], meta["U"]
    E_pad, N_eff = meta["E_pad"], meta["N_eff"]
    Gc = meta["g_per_core"]
    n_cores = meta["n_cores"]
    N_real = meta["N"]
    EB = U * 128
    NT = N_pad // 512
    assert N_pad % 512 == 0

    nc = bacc.Bacc("TRN2", target_bir_lowering=False, debug=False,
                   num_devices=n_cores)

    def param(name, shape, dtype=F32):
        return nc.dram_tensor(name, list(shape), dtype, kind="ExternalInput")

    x_slice = param("x_slice", (FIN + 1, N_pad))
    ea_t = param("ea_t", (4, E_pad))
    src_idx = param("src_idx", (R, 128, U), I32)
    ind_d = param("ind", (R * U, 128, 128))
    g_ind = param("g_ind", (R, 128, Gc))
    W_l1_aug = param("W_l1_aug", (FIN + 1, C))
    W_l2 = param("W_l2", (C, C))
    bn_g = param("bn_g", (C, 1))
    bn_b = param("bn_b", (C, 1))
    eps1 = param("eps1", (C, 1))
    Wi_in = param("Wi_in", (L, C, C))
    We1_aug = param("We1_aug", (L, 4, C))
    We2 = param("We2", (L, C, C))
    Wi_out = param("Wi_out", (L, C, C))
    bi_out = param("bi_out", (L, C, 1))
    delta = param("delta", (L, C, 1))
    const = param("const", (L, C, 1))
    W_r1 = param("W_r1", (C, 32))
    b_r1 = param("b_r1", (32, 1))
    W_r2 = param("W_r2", (32, 1))
    b_r2 = param("b_r2", (1, 1))

    y_out = nc.dram_tensor("y", [1, Gc], F32, kind="ExternalOutput")
    if debug_taps:
        dbg_h0 = nc.dram_tensor("dbg_h0", [C, N_pad], F32, kind="ExternalOutput")
        dbg_hb = nc.dram_tensor("dbg_hb", [N_eff, C], F32, kind="ExternalOutput")
        dbg_gat = nc.dram_tensor("dbg_gat", [128, U * 128], F32, kind="ExternalOutput")
        dbg_agg = nc.dram_tensor("dbg_agg", [C, N_pad], F32, kind="ExternalOutput")

    groups = [list(range(n_cores))]

    with tile.TileContext(nc) as tc, ExitStack() as ctx:
        pers = ctx.enter_context(tc.tile_pool(name="pers", bufs=1))
        wpool = ctx.enter_context(tc.tile_pool(name="wts", bufs=1))
        psA = ctx.enter_context(tc.tile_pool(name="psA", bufs=2, space="PSUM"))
        psB = ctx.enter_context(tc.tile_pool(name="psB", bufs=3, space="PSUM"))
        psAggP = ctx.enter_context(
            tc.tile_pool(name="psAgg", bufs=2, space="PSUM"))
        psGP = ctx.enter_context(tc.tile_pool(name="psG", bufs=1, space="PSUM"))
        work = ctx.enter_context(tc.tile_pool(name="work", bufs=2))
        small = ctx.enter_context(tc.tile_pool(name="small", bufs=2))
        dpool = ctx.enter_context(
            tc.tile_pool(name="dram", bufs=1, space="DRAM"))

        hb_bounce = dpool.tile([N_pad, C], F32, tag="hb_bounce")
        ar_in = dpool.tile([C, 2], F32, tag="ar_in")
        ar_out = dpool.tile([C, 2], F32, tag="ar_out", addr_space="Shared")

        h_loc = pers.tile([C, N_pad], F32, tag="h_loc")
        agg_T = pers.tile([C, N_pad], F32, tag="agg_T")

        def wtile(shape, tag, src_ap):
            t = wpool.tile(list(shape), F32, tag=tag)
            nc.sync.dma_start(t[:], src_ap)
            return t

        w_l1 = wtile((FIN + 1, C), "w_l1", W_l1_aug[:, :])
        w_l2 = wtile((C, C), "w_l2", W_l2[:, :])
        vec_bn_g = wtile((C, 1), "bn_g", bn_g[:, :])
        vec_bn_b = wtile((C, 1), "bn_b", bn_b[:, :])
        vec_eps1 = wtile((C, 1), "eps1", eps1[:, :])
        w_r1 = wtile((C, 32), "w_r1", W_r1[:, :])
        vec_br1 = wtile((32, 1), "b_r1", b_r1[:, :])
        w_r2 = wtile((32, 1), "w_r2", W_r2[:, :])
        vec_br2 = wtile((1, 1), "b_r2", b_r2[:, :])
        neg_ln2 = wpool.tile([128, 1], F32, tag="neg_ln2")
        nc.gpsimd.memset(neg_ln2[:], -LOG2)

        # ---------- embed + BN ----------
        with tc.tile_pool(name="embed", bufs=2) as epool, \
             tc.tile_pool(name="embed1", bufs=1) as epool1:
            h2_buf = h_loc
            stat_s = epool1.tile([C, NT], F32, tag="stat_s")
            stat_q = epool1.tile([C, NT], F32, tag="stat_q")
            for nt in range(NT):
                sl = slice(nt * 512, (nt + 1) * 512)
                xs = epool.tile([FIN + 1, 512], F32, tag="xs")
                nc.sync.dma_start(xs[:], x_slice[:, sl])
                ps1 = psA.tile([C, 512], F32, tag="b512")
                nc.tensor.matmul(ps1[:], w_l1[:], xs[:], start=True, stop=True)
                h1t = epool.tile([C, 512], F32, tag="h1t")
                nc.scalar.copy(h1t[:], ps1[:])
                ps2 = psA.tile([C, 512], F32, tag="b512")
                nc.tensor.matmul(ps2[:], w_l2[:], h1t[:], start=True,
                                 stop=True)
                nc.scalar.copy(h2_buf[:, sl], ps2[:])
                sq = epool.tile([C, 512], F32, tag="sq")
                nc.scalar.square(sq[:], ps2[:])
                nc.vector.tensor_reduce(
                    stat_s[:, nt:nt + 1], h2_buf[:, sl],
                    axis=mybir.AxisListType.X, op=ALU.add)
                nc.vector.tensor_reduce(
                    stat_q[:, nt:nt + 1], sq[:],
                    axis=mybir.AxisListType.X, op=ALU.add)
            st2 = epool1.tile([C, 2], F32, tag="st2")
            nc.vector.tensor_reduce(st2[:, 0:1], stat_s[:],
                                    axis=mybir.AxisListType.X, op=ALU.add)
            nc.vector.tensor_reduce(st2[:, 1:2], stat_q[:],
                                    axis=mybir.AxisListType.X, op=ALU.add)
            nc.sync.dma_start(ar_in[:], st2[:])
            nc.gpsimd.collective_compute(
                "AllReduce", ALU.add, replica_groups=groups,
                ins=[ar_in[:].opt()], outs=[ar_out[:].opt()])
            stg = epool1.tile([C, 2], F32, tag="stg")
            nc.sync.dma_start(stg[:], ar_out[:])
            mu = epool1.tile([C, 1], F32, tag="mu")
            nc.vector.tensor_scalar_mul(mu[:], stg[:, 0:1], 1.0 / N_real)
            ex2 = epool1.tile([C, 1], F32, tag="ex2")
            nc.vector.tensor_scalar_mul(ex2[:], stg[:, 1:2], 1.0 / N_real)
            musq = epool1.tile([C, 1], F32, tag="musq")
            nc.vector.tensor_mul(musq[:], mu[:], mu[:])
            var = epool1.tile([C, 1], F32, tag="var")
            nc.vector.tensor_sub(var[:], ex2[:], musq[:])
            nc.vector.tensor_scalar_add(var[:], var[:], BN_EPS)
            lnv = epool1.tile([C, 1], F32, tag="lnv")
            nc.scalar.activation(lnv[:], var[:], AF.Ln)
            rstd = epool1.tile([C, 1], F32, tag="rstd")
            nc.scalar.activation(rstd[:], lnv[:], AF.Exp, scale=-0.5)
            scl = epool1.tile([C, 1], F32, tag="scl")
            nc.vector.tensor_mul(scl[:], rstd[:], vec_bn_g[:])
            tmp = epool1.tile([C, 1], F32, tag="tmp")
            nc.vector.tensor_mul(tmp[:], mu[:], scl[:])
            shf = epool1.tile([C, 1], F32, tag="shf")
            nc.vector.tensor_sub(shf[:], vec_bn_b[:], tmp[:])
            nc.vector.tensor_add(shf[:], shf[:], vec_eps1[:])
            nc.scalar.activation(h_loc[:], h2_buf[:], AF.Identity,
                                 bias=shf[:], scale=scl[:])
        if debug_taps:
            nc.sync.dma_start(dbg_h0[:, :], h_loc[:])


        # ---------- interaction layers ----------
        for i in range(L):
            wi_in = wtile((C, C), "wi_in", Wi_in[i, :, :])
            we1 = wtile((4, C), "we1", We1_aug[i, :, :])
            we2 = wtile((C, C), "we2", We2[i, :, :])
            wi_out = wtile((C, C), "wi_out", Wi_out[i, :, :])
            v_biout = wtile((C, 1), "v_biout", bi_out[i, :, :])
            v_delta = wtile((C, 1), "v_delta", delta[i, :, :])
            v_const = wtile((C, 1), "v_const", const[i, :, :])

            hb_full = dpool.tile([N_eff, C], F32, tag=f"hb_full_{i}",
                                 addr_space="Shared")
            for nb in range(R):
                psH = psB.tile([128, C], F32, tag="bq")
                nc.tensor.matmul(psH[:], h_loc[:, nb * 128:(nb + 1) * 128],
                                 wi_in[:], start=True, stop=True)
                hbx = work.tile([128, C], F32, tag="hbx")
                nc.scalar.copy(hbx[:], psH[:])
                nc.sync.dma_start(
                    hb_bounce[:].rearrange("(b p) c -> b p c", p=128)[nb],
                    hbx[:])
            nc.gpsimd.collective_compute(
                "AllGather", ALU.bypass, replica_groups=groups,
                ins=[hb_bounce[:].opt()], outs=[hb_full[:].opt()])
            if debug_taps and i == 0:
                nc.sync.dma_start(dbg_hb[:, :], hb_full[:])

            for nb in range(R):
                idx_t = small.tile([128, U], I32, tag="idx")
                nc.sync.dma_start(idx_t[:], src_idx[nb, :, :])
                gat = work.tile([128, EB], F32, tag="gat")
                for j in range(U):
                    nc.gpsimd.indirect_dma_start(
                        out=gat[:, j * 128:(j + 1) * 128],
                        out_offset=None,
                        in_=hb_full[:],
                        in_offset=IndirectOffsetOnAxis(
                            ap=idx_t[:, j:j + 1], axis=0),
                    )
                if debug_taps and i == 0 and nb == 0:
                    nc.sync.dma_start(dbg_gat[:, :], gat[:])
                ea_tile = work.tile([4, EB], F32, tag="ea")
                nc.sync.dma_start(ea_tile[:], ea_t[:, nb * EB:(nb + 1) * EB])
                t_raw = work.tile([C, EB], F32, tag="t_raw")
                off = 0
                while off < EB:
                    w = min(512, EB - off)
                    psF1 = psA.tile([C, 512], F32, tag="b512")
                    nc.tensor.matmul(psF1[:, :w], we1[:],
                                     ea_tile[:, off:off + w],
                                     start=True, stop=True)
                    nc.scalar.activation(t_raw[:, off:off + w], psF1[:, :w],
                                         AF.Exp)
                    nc.scalar.activation(t_raw[:, off:off + w],
                                         t_raw[:, off:off + w], AF.Ln,
                                         bias=1.0)
                    off += w
                nc.vector.tensor_scalar_add(t_raw[:], t_raw[:], v_delta[:])
                for j in range(U):
                    js = slice(j * 128, (j + 1) * 128)
                    psF2 = psB.tile([128, C], F32, tag="bq")
                    nc.tensor.matmul(psF2[:], t_raw[:, js], we2[:],
                                     start=True, stop=True)
                    nc.vector.tensor_mul(gat[:, js], gat[:, js], psF2[:])
                ind_t = work.tile([128, EB], F32, tag="ind")
                nc.sync.dma_start(
                    ind_t[:].rearrange("p (u c) -> p u c", c=128),
                    ind_d.ap()[nb * U:(nb + 1) * U, :, :]
                    .rearrange("u p c -> p u c"))
                psAg = psAggP.tile([C, 128], F32, tag="agg")
                for j in range(U):
                    js = slice(j * 128, (j + 1) * 128)
                    nc.tensor.matmul(psAg[:], gat[:, js], ind_t[:, js],
                                     start=(j == 0), stop=(j == U - 1),
                                     skip_group_check=True)
                nc.scalar.copy(agg_T[:, nb * 128:(nb + 1) * 128], psAg[:])

            if debug_taps and i == 0:
                nc.sync.dma_start(dbg_agg[:, :], agg_T[:])
            for ut in range(NT):
                sl = slice(ut * 512, (ut + 1) * 512)
                psU = psA.tile([C, 512], F32, tag="b512")
                nc.tensor.matmul(psU[:], wi_out[:], agg_T[:, sl],
                                 start=True, stop=True)
                spu = work.tile([C, 512], F32, tag="spu")
                nc.scalar.activation(spu[:], psU[:], AF.Exp, bias=v_biout[:])
                nc.scalar.activation(spu[:], spu[:], AF.Ln, bias=1.0)
                nc.vector.tensor_add(h_loc[:, sl], h_loc[:, sl], spu[:])
                nc.vector.tensor_scalar_add(h_loc[:, sl], h_loc[:, sl],
                                            v_const[:])

        # ---------- readout ----------
        psG = psGP.tile([32, Gc], F32, tag="g")
        for nb in range(R):
            psP = psB.tile([128, 32], F32, tag="bq")
            nc.tensor.matmul(psP[:], h_loc[:, nb * 128:(nb + 1) * 128],
                             w_r1[:], start=True, stop=True)
            p_sb = small.tile([128, 32], F32, tag="p_sb")
            nc.scalar.copy(p_sb[:], psP[:])
            gi = small.tile([128, Gc], F32, tag="gi")
            nc.sync.dma_start(gi[:], g_ind[nb, :, :])
            nc.tensor.matmul(psG[:], p_sb[:], gi[:],
                             start=(nb == 0), stop=(nb == R - 1),
                             skip_group_check=True)
        r1 = small.tile([32, Gc], F32, tag="r1")
        nc.scalar.activation(r1[:], psG[:], AF.Exp, bias=vec_br1[:])
        nc.scalar.activation(r1[:], r1[:], AF.Ln, bias=1.0)
        r1b = small.tile([32, Gc], F32, tag="r1b")
        nc.scalar.activation(r1b[:], r1[:], AF.Identity,
                             bias=neg_ln2[:32, :])
        psR = psB.tile([1, Gc], F32, tag="bq")
        nc.tensor.matmul(psR[:], w_r2[:], r1b[:], start=True, stop=True)
        r2 = small.tile([1, Gc], F32, tag="r2")
        nc.scalar.activation(r2[:], psR[:], AF.Exp, bias=vec_br2[:])
        nc.scalar.activation(r2[:], r2[:], AF.Ln, bias=1.0)
        y_sb = small.tile([1, Gc], F32, tag="y_sb")
        nc.scalar.activation(y_sb[:], r2[:], AF.Identity, bias=neg_ln2[:1, :])
        nc.sync.dma_start(y_out[:, :], y_sb[:])

    nc.compile()
    return nc


# --------------------------------------------------------------------------
# runner
# --------------------------------------------------------------------------

_CACHE = {}


def _get_program(meta):
    key = tuple(sorted((k, v) for k, v in meta.items() if np.isscalar(v)))
    if key not in _CACHE:
        _CACHE[key] = _build(meta)
    return _CACHE[key]


_PARAM_NAMES = [
    "x_slice", "ea_t", "src_idx", "ind", "g_ind", "W_l1_aug", "W_l2",
    "bn_g", "bn_b", "eps1", "Wi_in", "We1_aug", "We2", "Wi_out", "bi_out",
    "delta", "const", "W_r1", "b_r1", "W_r2", "b_r2",
]


def run(inputs, trace=False, n_graphs=None):
    meta, percore = _prep(inputs, n_graphs=n_graphs)
    nc = _get_program(meta)
    in_maps = [{n: percore[k][n] for n in _PARAM_NAMES}
               for k in range(meta["n_cores"])]
    res = run_bass_kernel_spmd(nc, in_maps, list(range(meta["n_cores"])),
                               trace=trace)
    ys = [np.asarray(res.results[k]["y"]).reshape(-1)
          for k in range(meta["n_cores"])]
    y = np.concatenate(ys)[:meta["G"]].astype(np.float32)
    return y, res


def kernel(**inputs) -> np.ndarray:
    y, _ = run(inputs)
    return y
